# revision 2
# baseline (speedup 1.0000x reference)
"""AdaptiveFractalAnalysis distributed Trainium2 kernel (8 NeuronCores).

Strategy
--------
The reference computes three "fractal dimension" statistics of x [8192, 256]:
  - box-counting: pooled = avg_pool(x, s); count(pooled > pooled.mean()) per scale
  - correlation:  count(pairwise_dist(x) < s)  -> dominated by an 8192x8192x256 matmul
  - information:  histogram entropy of x per scale
then host-side slope fits and a softmax-weighted sum (scalar output).

Device split (uniform SPMD graph on 8 cores, no collectives -- final tiny
reduction happens on host):
  - cdist: d2 = sq_i + sq_j - 2 x@x.T. Using symmetry, the 16x16 grid of
    512-row blocks is covered once per unordered pair (136 pairs = 8 cores x 17).
    Per core the pairs are organized into "runs" sharing the lhs block so one
    PSUM group holds [128, 512*len(run)] and threshold counting amortizes.
    PSUM holds v = x@x.T - 0.5*sq_j (bf16 matmuls; sq_j via a K=2 ones-row
    matmul with bf16 hi/lo split). Count(d2 < t) == count(v > (sq_i - t)/2),
    per-partition thresholds. Counting runs on DVE (custom 2-threshold op,
    base-4096 packed exact counts) and ScalarE (Sign activation with
    per-partition bias + fused accumulation), greedily balanced.
  - box: pooled values for all scales computed transposed via matmul with a
    block-pooling matrix (partition = pooled column, free = row index), then
    one Sign-activation count per PSUM group with per-partition -theta bias.
  - hist: cumulative counts count(x < edge) for the deduped interior bin
    edges, on the core's own rows (f32, exact), split DVE/ACT.
Each counting instruction writes a per-partition accumulator into a column of
an SBUF "acc" tile; acc is DMA'd out and all decoding/slope math is numpy.
"""

import sys
import numpy as np

if "/opt/trn_rl_repo" not in sys.path:
    sys.path.insert(0, "/opt/trn_rl_repo")

import ml_dtypes

bf16 = ml_dtypes.bfloat16

N_ROWS, DIM = 8192, 256
NBLK = 16            # 512-row blocks
BLK = 512
NCORES = 8
B_PACK = 4096.0      # exact-int packing base for the 2-threshold DVE op
BIG = 3.0e38         # sentinel threshold: count(v > BIG) == 0

_BUILD_CACHE = {}
_CNT2 = None


# --------------------------------------------------------------------------
# custom DVE op: out = (x > c0) + (x > c1)*B ; accum_out = sum(out)
# --------------------------------------------------------------------------
def _register_cnt2():
    global _CNT2
    if _CNT2 is not None:
        return _CNT2
    import operator
    from concourse import dve_ops
    from concourse.dve_spec import Spec, Src0, C0, C1, C2, lower, _has_src1
    from concourse.dve_uop import DveOpSpec

    name = "CNT2_ANT_AFA"
    for o in dve_ops.OPS:
        if o.name == name:
            _CNT2 = o
            return o
    spec = Spec(
        body=(Src0 > C0) + (Src0 > C1) * C2,
        accum=operator.add,
        reference=lambda in0, in1, s0, s1, imm2: (
            (in0 > s0).astype(np.float32) + (in0 > s1).astype(np.float32) * imm2
        ),
    )
    row = dve_ops._CUSTOM_DVE_ROW_BASE + len(dve_ops.OPS)
    assert row < 0x20
    dve_ops._SUB_OPCODE_FOR_NAME[name] = row
    shas = {}
    for ver in ("v3",):
        uops = lower(spec, ver=ver)
        tmp = DveOpSpec(name=name, opcode=row, uops=uops, rd1_en=_has_src1(spec))
        shas[ver] = tmp.sha(ver)
    op = dve_ops.DveOp(name, spec, subdim=False, uops_sha=shas)
    dve_ops.OPS.append(op)
    dve_ops.CUSTOM_DVE_SPECS[name] = spec
    _CNT2 = op
    return op


# --------------------------------------------------------------------------
# pair assignment: cover all unordered block pairs, uniform per-core shape
# --------------------------------------------------------------------------
def _plan_runs():
    """Partition the 136 unordered block pairs into per-core runs.

    Every core gets the same run-length structure:
      offdiag runs of lengths OFF_STRUCT (pairs sharing the lhs block)
      + 2 diagonal single runs.
    Returns runs_per_core: list (len 8) of list of (a, [b...], is_diag).
    """
    # offdiag pairs per lhs row a: b in a+1..15 -> length 15-a
    OFF_STRUCT = (4, 4, 4, 2, 1)          # 15 offdiag pairs per core
    need = {4: 0, 2: 0, 1: 0}
    for s in OFF_STRUCT:
        need[s] += NCORES
    # cut rows (lengths 15,14,...,0) into chunks from the multiset `need`
    rows = [(a, list(range(a + 1, NBLK))) for a in range(NBLK)]
    chunks = {4: [], 2: [], 1: []}
    # greedy with small backtracking: take largest needed chunk that fits
    rows_sorted = sorted(rows, key=lambda r: -len(r[1]))
    for a, bs in rows_sorted:
        i = 0
        rem = bs
        while rem:
            for size in (4, 2, 1):
                if len(chunks[size]) < need[size] and len(rem) >= size:
                    chunks[size].append((a, rem[:size]))
                    rem = rem[size:]
                    break
            else:
                # force split into singles if mismatch (shouldn't happen with
                # counts below, but keep safe)
                chunks[1].append((a, rem[:1]))
                rem = rem[1:]
    ok = all(len(chunks[s]) == need[s] for s in (4, 2, 1))
    if not ok:
        # fallback: all doubles + singles structure (always feasible)
        OFF_STRUCT = (2, 2, 2, 2, 2, 2, 2, 1)
        need = {4: 0, 2: 0, 1: 0}
        for s in OFF_STRUCT:
            need[s] += NCORES
        chunks = {4: [], 2: [], 1: []}
        for a, bs in rows_sorted:
            rem = list(bs)
            while rem:
                for size in (2, 1):
                    if len(chunks[size]) < need[size] and len(rem) >= size:
                        chunks[size].append((a, rem[:size]))
                        rem = rem[size:]
                        break
                else:
                    chunks[1].append((a, rem[:1]))
                    rem = rem[1:]
        assert all(len(chunks[s]) == need[s] for s in (4, 2, 1)), (
            {k: len(v) for k, v in chunks.items()})
    runs_per_core = []
    for c in range(NCORES):
        runs = []
        for s in OFF_STRUCT:
            a, bs = chunks[s].pop()
            runs.append((a, bs, False))
        runs.append((2 * c, [2 * c], True))
        runs.append((2 * c + 1, [2 * c + 1], True))
        runs_per_core.append(runs)
    return OFF_STRUCT, runs_per_core


# --------------------------------------------------------------------------
# build the bass kernel for a given (u, E, box-structure) config
# --------------------------------------------------------------------------
def _build(cfg_key, u, n_tp, edges, box_groups, run_struct):
    """run_struct: tuple of run lengths incl 2 diag singles, same every core.
    box_groups: list of group sizes (#pooled columns per PSUM group), <=128.
    Returns (nc, meta) where meta describes acc slot layout.
    """
    from concourse import bacc, tile, mybir

    CNT2 = _register_cnt2()
    f32 = mybir.dt.float32
    bt = mybir.dt.bfloat16
    AT = mybir.ActivationFunctionType
    ALU = mybir.AluOpType

    n_runs = len(run_struct)
    n_pairs = sum(run_struct)
    assert n_pairs == 17
    E = len(edges)
    NG = len(box_groups)
    MTOT = sum(box_groups)

    # ---- engine cost model (ns) for greedy balancing of count passes ----
    def dve_cost(w):      # CNT2 (2 thresholds) over [128, w] psum/sbuf 1x
        return (140 + w) / 0.96
    def act_cost(w):      # Sign+accum (1 threshold), psum
        return (180 + w) / 1.2 + 185

    nc = bacc.Bacc("TRN2", target_bir_lowering=False, debug=False,
                   num_devices=NCORES)
    dL = nc.dram_tensor("L", [2, n_runs, 128, BLK], bt, kind="ExternalInput")
    dR = nc.dram_tensor("R", [2, n_pairs, 128, BLK], bt, kind="ExternalInput")
    dNSQ = nc.dram_tensor("NSQ", [n_pairs, 2, BLK], bt, kind="ExternalInput")
    dCIK = nc.dram_tensor("CIK", [128, n_runs * 4 * u], f32, kind="ExternalInput")
    dXF = nc.dram_tensor("XF", [128, 2048], f32, kind="ExternalInput")
    dBX = nc.dram_tensor("BX", [2, 128, 1024], bt, kind="ExternalInput")
    dPM = nc.dram_tensor("PM", [2, 128, max(MTOT, 1)], bt, kind="ExternalInput")
    dBTH = nc.dram_tensor("BTH", [128, max(NG, 1)], f32, kind="ExternalInput")
    dHED = nc.dram_tensor("HED", [128, max(E, 1)], f32, kind="ExternalInput")
    NSLOT = 512
    dOUT = nc.dram_tensor("OUT", [128, NSLOT], f32, kind="ExternalOutput")

    meta = {"cdist": [], "box": [], "hist": [], "w": {}}
    slot_ctr = [0]

    def new_slot():
        s = slot_ctr[0]
        slot_ctr[0] += 1
        assert s < NSLOT
        return s

    # threshold pairs for the DVE op / ACT passes: list of (k_lo, k_hi|None)
    tps = []
    k = 0
    while k < u:
        if k + 1 < u:
            tps.append((k, k + 1))
        else:
            tps.append((k, None))
        k += 2
    assert len(tps) == n_tp

    eng_load = {"dve": 0.0, "act": 0.0}

    with tile.TileContext(nc) as tc:
        import contextlib
        ctx = contextlib.ExitStack()
        with ctx:
            const_p = ctx.enter_context(tc.tile_pool(name="const", bufs=1))
            acc = const_p.tile([128, NSLOT], f32)
            cik = const_p.tile([128, n_runs * 4 * u], f32)
            nc.sync.dma_start(cik[:], dCIK[:])
            xf = const_p.tile([128, 2048], f32)
            nc.sync.dma_start(xf[:], dXF[:])
            ones2 = const_p.tile([2, 128], bt)
            nc.vector.memset(ones2[:], 1.0)
            scr = const_p.tile([128, 2048], bt)   # dve scratch out
            scrf = const_p.tile([128, 2048], f32)  # act scratch out
            hed = const_p.tile([128, max(E, 1)], f32)
            nc.sync.dma_start(hed[:], dHED[:])

            # ---------------- box counting (own psum pool, freed after) ----
            if MTOT > 0:
                with tc.tile_pool(name="boxps", bufs=1, space="PSUM") as boxps, \
                     tc.tile_pool(name="boxsb", bufs=1) as boxsb:
                    bx0 = boxsb.tile([128, 1024], bt)
                    nc.sync.dma_start(bx0[:], dBX[0])
                    bx1 = boxsb.tile([128, 1024], bt)
                    nc.sync.dma_start(bx1[:], dBX[1])
                    pm0 = boxsb.tile([128, MTOT], bt)
                    nc.sync.dma_start(pm0[:], dPM[0, :, 0:MTOT])
                    pm1 = boxsb.tile([128, MTOT], bt)
                    nc.sync.dma_start(pm1[:], dPM[1, :, 0:MTOT])
                    bth = boxsb.tile([128, NG], f32)
                    nc.sync.dma_start(bth[:], dBTH[:, 0:NG])
                    g0 = 0
                    for g, mg in enumerate(box_groups):
                        bps = boxps.tile([128, 1024], f32, tag="bps")
                        for nsl in range(2):
                            nc.tensor.matmul(
                                bps[0:mg, nsl * 512:(nsl + 1) * 512],
                                pm0[:, g0:g0 + mg],
                                bx0[:, nsl * 512:(nsl + 1) * 512],
                                start=True, stop=False)
                            nc.tensor.matmul(
                                bps[0:mg, nsl * 512:(nsl + 1) * 512],
                                pm1[:, g0:g0 + mg],
                                bx1[:, nsl * 512:(nsl + 1) * 512],
                                start=False, stop=True)
                        slot = new_slot()
                        # count(pooled > theta) per partition over 1024 cols:
                        # sign(pooled - theta); bth holds -theta
                        nc.scalar.activation(
                            scrf[0:mg, 0:1024], bps[0:mg, 0:1024], AT.Sign,
                            bias=bth[0:mg, g:g + 1], scale=1.0,
                            accum_out=acc[0:mg, slot:slot + 1])
                        meta["box"].append((slot, g, mg, 1024))
                        g0 += mg

            # ---------------- hist counting (greedy DVE/ACT split) --------
            ei = 0
            while ei < E:
                if ei + 1 < E and \
                   eng_load["dve"] + dve_cost(2048) <= eng_load["act"] + 2 * act_cost(2048):
                    slot = new_slot()
                    nc.vector._custom_dve(
                        CNT2, out=scr[:, 0:2048], in0=xf[:],
                        s0=float(edges[ei]), s1=float(edges[ei + 1]),
                        imm2=B_PACK, accum_out=acc[:, slot:slot + 1])
                    meta["hist"].append(("dve", slot, ei, ei + 1, 2048))
                    eng_load["dve"] += dve_cost(2048)
                    ei += 2
                else:
                    slot = new_slot()
                    # count(x < e): sign(e - x) => scale=-1, bias=e
                    nc.scalar.activation(
                        scrf[:, 0:2048], xf[:], AT.Sign,
                        bias=hed[:, ei:ei + 1], scale=-1.0,
                        accum_out=acc[:, slot:slot + 1])
                    meta["hist"].append(("act", slot, ei, None, 2048))
                    eng_load["act"] += act_cost(2048)
                    ei += 1

            # ---------------- cdist ---------------------------------------
            psum_p = ctx.enter_context(
                tc.tile_pool(name="cps", bufs=2, space="PSUM"))
            lp = ctx.enter_context(tc.tile_pool(name="lp", bufs=3))
            rp = ctx.enter_context(tc.tile_pool(name="rp", bufs=8))
            np_ = ctx.enter_context(tc.tile_pool(name="nsqp", bufs=8))

            pair_slot = 0
            for ri, rl in enumerate(run_struct):
                w = rl * BLK
                l0 = lp.tile([128, BLK], bt, tag="l0")
                nc.sync.dma_start(l0[:], dL[0, ri])
                l1 = lp.tile([128, BLK], bt, tag="l1")
                nc.sync.dma_start(l1[:], dL[1, ri])
                rts = []
                nsqs = []
                for j in range(rl):
                    r0 = rp.tile([128, BLK], bt, tag="r0")
                    nc.sync.dma_start(r0[:], dR[0, pair_slot + j])
                    r1 = rp.tile([128, BLK], bt, tag="r1")
                    nc.sync.dma_start(r1[:], dR[1, pair_slot + j])
                    rts.append((r0, r1))
                    nst = np_.tile([2, BLK], bt, tag="nst")
                    nc.sync.dma_start(nst[:], dNSQ[pair_slot + j])
                    nsqs.append(nst)
                for r in range(4):
                    pg = psum_p.tile([128, 2048], f32, tag="pg")
                    for kt in range(2):
                        lt = (l0, l1)[kt]
                        for j in range(rl):
                            nc.tensor.matmul(
                                pg[:, j * BLK:(j + 1) * BLK],
                                lt[:, r * 128:(r + 1) * 128],
                                rts[j][kt][:],
                                start=(kt == 0), stop=False)
                    for j in range(rl):
                        nc.tensor.matmul(
                            pg[:, j * BLK:(j + 1) * BLK],
                            ones2[:], nsqs[j][:],
                            start=False, stop=True)
                    # counting
                    for (ka, kb) in tps:
                        ca = cik[:, (ri * 4 + r) * u + ka:(ri * 4 + r) * u + ka + 1]
                        use_dve = (
                            eng_load["dve"] + dve_cost(w)
                            <= eng_load["act"]
                            + (2 if kb is not None else 1) * act_cost(w))
                        if use_dve:
                            slot = new_slot()
                            if kb is not None:
                                cb = cik[:, (ri * 4 + r) * u + kb:
                                         (ri * 4 + r) * u + kb + 1]
                            else:
                                cb = BIG
                            nc.vector._custom_dve(
                                CNT2, out=scr[:, 0:w], in0=pg[:, 0:w],
                                s0=ca, s1=cb, imm2=B_PACK,
                                accum_out=acc[:, slot:slot + 1])
                            meta["cdist"].append(
                                ("dve", slot, ri, r, ka, kb, w))
                            eng_load["dve"] += dve_cost(w)
                        else:
                            slot = new_slot()
                            # count(v > c): sign(-v + c) -> (w - sum)/2
                            nc.scalar.activation(
                                scrf[:, 0:w], pg[:, 0:w], AT.Sign,
                                bias=ca, scale=-1.0,
                                accum_out=acc[:, slot:slot + 1])
                            meta["cdist"].append(
                                ("act", slot, ri, r, ka, None, w))
                            eng_load["act"] += act_cost(w)
                            if kb is not None:
                                slot = new_slot()
                                cbap = cik[:, (ri * 4 + r) * u + kb:
                                           (ri * 4 + r) * u + kb + 1]
                                nc.scalar.activation(
                                    scrf[:, 0:w], pg[:, 0:w], AT.Sign,
                                    bias=cbap, scale=-1.0,
                                    accum_out=acc[:, slot:slot + 1])
                                meta["cdist"].append(
                                    ("act", slot, ri, r, kb, None, w))
                                eng_load["act"] += act_cost(w)
                pair_slot += rl

            nc.sync.dma_start(dOUT[:], acc[:])

    nc.compile()
    meta["eng_load"] = dict(eng_load)
    return nc, meta


# --------------------------------------------------------------------------
# host orchestration
# --------------------------------------------------------------------------
def kernel(x, scale_params, scale_importance):
    from concourse.bass_utils import run_bass_kernel_spmd

    x = np.asarray(x, dtype=np.float32)
    scale_params = np.asarray(scale_params, dtype=np.float32)
    scale_importance = np.asarray(scale_importance, dtype=np.float32)
    n, d = x.shape
    assert (n, d) == (N_ROWS, DIM)

    x64 = x.astype(np.float64)
    # ---- dynamic scales (mirror reference host-side computation) ----
    s = np.exp(scale_params.astype(np.float64))
    std_factor = float(x64.std(ddof=1) / x64.mean())
    std_factor = min(max(std_factor, 0.5), 2.0)
    adj = np.clip(s * std_factor, 2.0, 16.0)
    scales = [int(v) for v in adj]
    log_s = np.log(np.asarray(scales, np.float32)).astype(np.float64)

    # ---- derived constants ----
    uniq_scales = sorted(set(scales))
    uniq_t = sorted(set(float(ss) * float(ss) for ss in scales))
    u = len(uniq_t)
    n_tp = (u + 1) // 2

    # box: theta per unique scale; pooling matrix columns
    box_cols = []   # list of (scale, block_index)
    thetas = {}
    for ss in uniq_scales:
        m = d // ss
        nn = m * ss
        thetas[ss] = float(x64[:, :nn].sum() / (n * nn))
        for b in range(m):
            box_cols.append((ss, b))
    MTOT = len(box_cols)
    box_groups = []
    rem = MTOT
    while rem > 0:
        g = min(128, rem)
        box_groups.append(g)
        rem -= g
    NG = len(box_groups)

    # hist: deduped interior edges (f32 linspace like jnp.histogram)
    xmin = float(x.min())
    xmax = float(x.max())
    edge_list = []      # deduped values
    edge_map = {}       # (scale, k) -> index into edge_list
    for ss in uniq_scales:
        ed = np.linspace(np.float32(xmin), np.float32(xmax), ss + 1,
                         dtype=np.float32)
        for kk in range(1, ss):
            v = float(ed[kk])
            if v not in edge_map:
                edge_map[v] = len(edge_list)
                edge_list.append(v)
            edge_map[(ss, kk)] = edge_map[v]
    E = len(edge_list)

    run_struct_off, runs_per_core = _plan_runs()
    run_struct = tuple(list(run_struct_off) + [1, 1])

    cfg_key = (u, n_tp, E, tuple(box_groups), run_struct, MTOT)
    if cfg_key not in _BUILD_CACHE:
        _BUILD_CACHE[cfg_key] = _build(
            cfg_key, u, n_tp, edge_list, box_groups, run_struct)
    nc, meta = _BUILD_CACHE[cfg_key]

    # ---- per-core inputs ----
    xb = x.astype(bf16)                       # [8192, 256]
    xTb = np.ascontiguousarray(xb.T)          # [256, 8192]
    sq = (x.astype(np.float32) ** 2).sum(axis=1, dtype=np.float32)  # [8192]
    nsq_half = -0.5 * sq
    nsq_hi = nsq_half.astype(bf16)
    nsq_lo = (nsq_half - nsq_hi.astype(np.float32)).astype(bf16)

    n_runs = len(run_struct)
    n_pairs = 17

    # pooling matrix [256, MTOT] bf16 (same all cores)
    PM = np.zeros((256, max(MTOT, 1)), np.float32)
    for col, (ss, b) in enumerate(box_cols):
        PM[b * ss:(b + 1) * ss, col] = 1.0 / ss
    PM_b = PM.astype(bf16)
    dPM = np.stack([PM_b[0:128], PM_b[128:256]])          # [2,128,MTOT]
    dBTH = np.zeros((128, max(NG, 1)), np.float32)
    g0 = 0
    for g, mg in enumerate(box_groups):
        for p in range(mg):
            ss, b = box_cols[g0 + p]
            dBTH[p, g] = -thetas[ss]
        g0 += mg

    t_arr = np.asarray(uniq_t, np.float64)
    dHED_np = np.zeros((128, max(E, 1)), np.float32)
    for ei2, ev in enumerate(edge_list):
        dHED_np[:, ei2] = ev

    in_maps = []
    core_meta = []
    for c in range(NCORES):
        runs = runs_per_core[c]
        L = np.zeros((2, n_runs, 128, BLK), bf16)
        R = np.zeros((2, n_pairs, 128, BLK), bf16)
        NSQ = np.zeros((n_pairs, 2, BLK), bf16)
        CIK = np.zeros((128, n_runs * 4 * u), np.float32)
        ps = 0
        pair_list = []
        for ri, (a, bs, is_diag) in enumerate(runs):
            for kt in range(2):
                L[kt, ri] = xTb[kt * 128:(kt + 1) * 128,
                                a * BLK:(a + 1) * BLK]
            for j, b in enumerate(bs):
                for kt in range(2):
                    R[kt, ps + j] = xTb[kt * 128:(kt + 1) * 128,
                                        b * BLK:(b + 1) * BLK]
                NSQ[ps + j, 0] = nsq_hi[b * BLK:(b + 1) * BLK]
                NSQ[ps + j, 1] = nsq_lo[b * BLK:(b + 1) * BLK]
            for r in range(4):
                i0 = a * BLK + r * 128
                sqi = sq[i0:i0 + 128].astype(np.float64)
                for kk in range(u):
                    CIK[:, (ri * 4 + r) * u + kk] = (
                        (sqi - t_arr[kk]) * 0.5).astype(np.float32)
            pair_list.append((a, list(bs), is_diag))
            ps += len(bs)
        rows = x[c * 1024:(c + 1) * 1024]                  # own rows
        XF = np.ascontiguousarray(
            rows.reshape(8, 128, 256).transpose(1, 0, 2).reshape(128, 2048))
        rowsT_b = xTb[:, c * 1024:(c + 1) * 1024]
        BX = np.stack([rowsT_b[0:128], rowsT_b[128:256]])  # [2,128,1024]
        in_maps.append({
            "L": L, "R": R, "NSQ": NSQ, "CIK": CIK,
            "XF": np.ascontiguousarray(XF),
            "BX": np.ascontiguousarray(BX),
            "PM": dPM, "BTH": dBTH, "HED": dHED_np,
        })
        core_meta.append(pair_list)

    res = run_bass_kernel_spmd(nc, in_maps, core_ids=list(range(NCORES)))

    # ---- decode ----
    corr_counts = np.zeros(u, np.float64)
    box_counts = {ss: 0.0 for ss in uniq_scales}
    hist_cum = np.zeros(E, np.float64)

    for c in range(NCORES):
        out = res.results[c]["OUT"].astype(np.float64)   # [128, NSLOT]
        pair_list = core_meta[c]
        for ent in meta["cdist"]:
            kind, slot, ri, r, ka, kb, w = ent
            a, bs, is_diag = pair_list[ri]
            wt = 1.0 if is_diag else 2.0
            vals = out[:, slot]
            if kind == "dve":
                c1 = np.mod(vals, B_PACK)
                c2 = np.floor(vals / B_PACK)
                corr_counts[ka] += wt * c1.sum()
                if kb is not None:
                    corr_counts[kb] += wt * c2.sum()
            else:
                # count(v > c) = (w - sum_sign)/2 per partition
                corr_counts[ka] += wt * ((w - vals) / 2.0).sum()
        for (slot, g, mg, wbox) in meta["box"]:
            vals = out[0:mg, slot]
            cnt = (wbox + vals) / 2.0     # count(pooled > theta)
            gg0 = sum(box_groups[:g])
            for p in range(mg):
                ss, b = box_cols[gg0 + p]
                box_counts[ss] += cnt[p]
        for ent in meta["hist"]:
            kind, slot, ea, eb, wh = ent
            vals = out[:, slot]
            if kind == "dve":
                cgt1 = np.mod(vals, B_PACK).sum()
                cgt2 = np.floor(vals / B_PACK).sum()
                hist_cum[ea] += 2048 * 128 - cgt1   # count(x < e) = w - count(x > e)  (ties ~0)
                hist_cum[eb] += 2048 * 128 - cgt2
            else:
                hist_cum[ea] += ((wh + vals) / 2.0).sum()

    # ---- slope fits (host) ----
    def slope(xv, yv):
        xv = np.asarray(xv, np.float64)
        yv = np.asarray(yv, np.float64)
        xm = xv.mean()
        ym = yv.mean()
        dx = xv - xm
        with np.errstate(divide="ignore", invalid="ignore"):
            return float((dx * (yv - ym)).sum() / (dx * dx).sum())

    t_index = {t: i for i, t in enumerate(uniq_t)}
    corr_per_scale = np.array(
        [corr_counts[t_index[float(ss) * float(ss)]] for ss in scales])
    box_per_scale = np.array([box_counts[ss] for ss in scales])

    ents = []
    total = float(n * d)
    for ss in scales:
        cum = np.zeros(ss + 1, np.float64)
        cum[0] = 0.0
        cum[ss] = total
        for kk in range(1, ss):
            cum[kk] = hist_cum[edge_map[(ss, kk)]]
        hist = np.diff(cum)
        p = hist / total
        with np.errstate(divide="ignore", invalid="ignore"):
            ents.append(float(-(np.where(p > 0, p * np.log(
                np.where(p > 0, p, 1.0)), 0.0)).sum()))

    with np.errstate(divide="ignore", invalid="ignore"):
        box_dim = -slope(log_s, np.log(box_per_scale))
        corr_dim = slope(log_s, np.log(corr_per_scale))
    info_dim = slope(log_s, np.asarray(ents))

    # softmax in f32 like the reference
    si = scale_importance.astype(np.float64)
    w_ = np.exp(si - si.max())
    w_ = w_ / w_.sum()
    out_val = w_[0] * box_dim + w_[1] * corr_dim + w_[2] * info_dim
    return np.float32(out_val)


# revision 5
# speedup vs baseline: 1.0283x; 1.0283x over previous
"""AdaptiveFractalAnalysis distributed Trainium2 kernel (8 NeuronCores).

Strategy
--------
The reference computes three "fractal dimension" statistics of x [8192, 256]:
  - box-counting: pooled = avg_pool(x, s); count(pooled > pooled.mean()) per scale
  - correlation:  count(pairwise_dist(x) < s)  -> dominated by an 8192x8192x256 matmul
  - information:  histogram entropy of x per scale
then host-side slope fits and a softmax-weighted sum (scalar output).

Device split (uniform SPMD graph on 8 cores, no collectives -- final tiny
reduction happens on host):
  - cdist: d2 = sq_i + sq_j - 2 x@x.T. Using symmetry, the 16x16 grid of
    512-row blocks is covered once per unordered pair (136 pairs = 8 cores x 17).
    Per core the pairs are organized into "runs" sharing the lhs block so one
    PSUM group holds [128, 512*len(run)] and threshold counting amortizes.
    PSUM holds v = x@x.T - 0.5*sq_j (bf16 matmuls; sq_j via a K=2 ones-row
    matmul with bf16 hi/lo split). Count(d2 < t) == count(v > (sq_i - t)/2),
    per-partition thresholds. Counting runs on DVE (custom 2-threshold op,
    base-4096 packed exact counts) and ScalarE (Sign activation with
    per-partition bias + fused accumulation), greedily balanced.
  - box: pooled values for all scales computed transposed via matmul with a
    block-pooling matrix (partition = pooled column, free = row index), then
    one Sign-activation count per PSUM group with per-partition -theta bias.
  - hist: cumulative counts count(x < edge) for the deduped interior bin
    edges, on the core's own rows (f32, exact), split DVE/ACT.
Each counting instruction writes a per-partition accumulator into a column of
an SBUF "acc" tile; acc is DMA'd out and all decoding/slope math is numpy.
"""

import sys
import numpy as np

if "/opt/trn_rl_repo" not in sys.path:
    sys.path.insert(0, "/opt/trn_rl_repo")

import ml_dtypes

bf16 = ml_dtypes.bfloat16

N_ROWS, DIM = 8192, 256
NBLK = 16            # 512-row blocks
BLK = 512
NCORES = 8
B_PACK = 4096.0      # exact-int packing base for the 2-threshold DVE op
BIG = 3.0e38         # sentinel threshold: count(v > BIG) == 0

_BUILD_CACHE = {}
_CNT2 = None


# --------------------------------------------------------------------------
# custom DVE op: out = (x > c0) + (x > c1)*B ; accum_out = sum(out)
# --------------------------------------------------------------------------
def _register_cnt2():
    global _CNT2
    if _CNT2 is not None:
        return _CNT2
    import operator
    from concourse import dve_ops
    from concourse.dve_spec import Spec, Src0, C0, C1, C2, lower, _has_src1
    from concourse.dve_uop import DveOpSpec

    name = "CNT2_ANT_AFA"
    for o in dve_ops.OPS:
        if o.name == name:
            _CNT2 = o
            return o
    spec = Spec(
        body=(Src0 > C0) + (Src0 > C1) * C2,
        accum=operator.add,
        reference=lambda in0, in1, s0, s1, imm2: (
            (in0 > s0).astype(np.float32) + (in0 > s1).astype(np.float32) * imm2
        ),
    )
    row = dve_ops._CUSTOM_DVE_ROW_BASE + len(dve_ops.OPS)
    assert row < 0x20
    dve_ops._SUB_OPCODE_FOR_NAME[name] = row
    shas = {}
    for ver in ("v3",):
        uops = lower(spec, ver=ver)
        tmp = DveOpSpec(name=name, opcode=row, uops=uops, rd1_en=_has_src1(spec))
        shas[ver] = tmp.sha(ver)
    op = dve_ops.DveOp(name, spec, subdim=False, uops_sha=shas)
    dve_ops.OPS.append(op)
    dve_ops.CUSTOM_DVE_SPECS[name] = spec
    _CNT2 = op
    return op


# --------------------------------------------------------------------------
# pair assignment: cover all unordered block pairs, uniform per-core shape
# --------------------------------------------------------------------------
def _plan_runs():
    """Partition the 136 unordered block pairs into per-core runs.

    Every core gets the same run-length structure:
      offdiag runs of lengths OFF_STRUCT (pairs sharing the lhs block)
      + 2 diagonal single runs.
    Returns runs_per_core: list (len 8) of list of (a, [b...], is_diag).
    """
    # offdiag pairs per lhs row a: b in a+1..15 -> length 15-a
    OFF_STRUCT = (4, 4, 4, 2, 1)          # 15 offdiag pairs per core
    need = {4: 0, 2: 0, 1: 0}
    for s in OFF_STRUCT:
        need[s] += NCORES
    # cut rows (lengths 15,14,...,0) into chunks from the multiset `need`
    rows = [(a, list(range(a + 1, NBLK))) for a in range(NBLK)]
    chunks = {4: [], 2: [], 1: []}
    # greedy with small backtracking: take largest needed chunk that fits
    rows_sorted = sorted(rows, key=lambda r: -len(r[1]))
    for a, bs in rows_sorted:
        i = 0
        rem = bs
        while rem:
            for size in (4, 2, 1):
                if len(chunks[size]) < need[size] and len(rem) >= size:
                    chunks[size].append((a, rem[:size]))
                    rem = rem[size:]
                    break
            else:
                # force split into singles if mismatch (shouldn't happen with
                # counts below, but keep safe)
                chunks[1].append((a, rem[:1]))
                rem = rem[1:]
    ok = all(len(chunks[s]) == need[s] for s in (4, 2, 1))
    if not ok:
        # fallback: all doubles + singles structure (always feasible)
        OFF_STRUCT = (2, 2, 2, 2, 2, 2, 2, 1)
        need = {4: 0, 2: 0, 1: 0}
        for s in OFF_STRUCT:
            need[s] += NCORES
        chunks = {4: [], 2: [], 1: []}
        for a, bs in rows_sorted:
            rem = list(bs)
            while rem:
                for size in (2, 1):
                    if len(chunks[size]) < need[size] and len(rem) >= size:
                        chunks[size].append((a, rem[:size]))
                        rem = rem[size:]
                        break
                else:
                    chunks[1].append((a, rem[:1]))
                    rem = rem[1:]
        assert all(len(chunks[s]) == need[s] for s in (4, 2, 1)), (
            {k: len(v) for k, v in chunks.items()})
    runs_per_core = []
    for c in range(NCORES):
        runs = []
        for s in OFF_STRUCT:
            a, bs = chunks[s].pop()
            runs.append((a, bs, False))
        runs.append((2 * c, [2 * c], True))
        runs.append((2 * c + 1, [2 * c + 1], True))
        runs_per_core.append(runs)
    return OFF_STRUCT, runs_per_core


# --------------------------------------------------------------------------
# build the bass kernel for a given (u, E, box-structure) config
# --------------------------------------------------------------------------
def _build(cfg_key, u, n_tp, edges, box_groups, run_struct):
    """run_struct: tuple of run lengths incl 2 diag singles, same every core.
    box_groups: list of group sizes (#pooled columns per PSUM group), <=128.
    Returns (nc, meta) where meta describes acc slot layout.
    """
    from concourse import bacc, tile, mybir

    CNT2 = _register_cnt2()
    f32 = mybir.dt.float32
    bt = mybir.dt.bfloat16
    AT = mybir.ActivationFunctionType
    ALU = mybir.AluOpType

    n_runs = len(run_struct)
    n_pairs = sum(run_struct)
    assert n_pairs == 17
    E = len(edges)
    NG = len(box_groups)
    MTOT = sum(box_groups)

    # ---- engine cost model (ns) for balancing count passes ----
    def dve_cost(w):      # CNT2 (2 thresholds) over [128, w] psum/sbuf 1x
        return (140 + w) / 0.96

    def act_cost(w):      # Sign+accum (1 threshold)
        return (180 + w) / 1.2 + 185

    def plan_group(w, tp_list):
        """Balance tp_list (list of (ka, kb|None)) across DVE/ACT for one
        psum group of width w. Returns ops: (engine, ka, kb, c0, c1) with
        column range [c0, c1). Exhaustive over assignments + one column
        split of one tp."""
        best = None
        ntp = len(tp_list)
        for mask in range(1 << ntp):
            base_d = [t for i, t in enumerate(tp_list) if (mask >> i) & 1]
            base_a = [t for i, t in enumerate(tp_list) if not (mask >> i) & 1]
            for split_i in range(-1, ntp):
                dload = sum(dve_cost(w) for t in base_d)
                aload = sum(act_cost(w) * (2 if t[1] is not None else 1)
                            for t in base_a)
                ops = [("dve", t[0], t[1], 0, w) for t in base_d]
                ops += [("act", t[0], t[1], 0, w) for t in base_a]
                if split_i >= 0:
                    t = tp_list[split_i]
                    on_dve = (mask >> split_i) & 1
                    # move columns of t from its engine to the other
                    nth = 2 if t[1] is not None else 1
                    bestf, bestm = None, None
                    for fi in range(0, 17):
                        f = fi / 16.0
                        wd = int(w * f) // 16 * 16
                        wa = w - wd
                        dl = dload + (dve_cost(wd) if wd > 0 else 0)                             - (dve_cost(w) if on_dve else 0)
                        al = aload + (nth * act_cost(wa) if wa > 0 else 0)                             - (nth * act_cost(w) if not on_dve else 0)
                        m = max(dl, al)
                        if bestm is None or m < bestm:
                            bestm, bestf = m, (wd, wa)
                    wd, wa = bestf
                    ops2 = [o for o in ops if not (o[1] == t[0] and o[3] == 0 and o[4] == w and ((o[0] == "dve") == bool(on_dve)))]
                    # rebuild: remove the split tp's full op, add partials
                    ops2 = []
                    for o in ops:
                        eng, ka, kb, c0, c1 = o
                        if ka == t[0] and kb == t[1] and (
                                (eng == "dve") == bool(on_dve)):
                            continue
                        ops2.append(o)
                    if wd > 0:
                        ops2.append(("dve", t[0], t[1], 0, wd))
                    if wa > 0:
                        ops2.append(("act", t[0], t[1], wd, w))
                    m = bestm
                    cand = (m, ops2)
                else:
                    cand = (max(dload, aload), ops)
                if best is None or cand[0] < best[0]:
                    best = cand
        return best[1], best[0]

    nc = bacc.Bacc("TRN2", target_bir_lowering=False, debug=False,
                   num_devices=NCORES)
    dL = nc.dram_tensor("L", [2, n_runs, 128, BLK], bt, kind="ExternalInput")
    dR = nc.dram_tensor("R", [2, n_runs, 128, 2048], bt, kind="ExternalInput")
    dNSQ = nc.dram_tensor("NSQ", [n_runs, 2, 2048], bt, kind="ExternalInput")
    dCIK = nc.dram_tensor("CIK", [128, n_runs * 4 * u], f32, kind="ExternalInput")
    dXF = nc.dram_tensor("XF", [128, 2048], f32, kind="ExternalInput")
    dBX = nc.dram_tensor("BX", [2, 128, 1024], bt, kind="ExternalInput")
    dPM = nc.dram_tensor("PM", [2, 128, max(MTOT, 1)], bt, kind="ExternalInput")
    dBTH = nc.dram_tensor("BTH", [128, max(NG, 1)], f32, kind="ExternalInput")
    dHED = nc.dram_tensor("HED", [128, max(E, 1)], f32, kind="ExternalInput")
    NSLOT = 512
    dOUT = nc.dram_tensor("OUT", [128, NSLOT], f32, kind="ExternalOutput")

    meta = {"cdist": [], "box": [], "hist": []}
    slot_ctr = [0]

    def new_slot():
        sl = slot_ctr[0]
        slot_ctr[0] += 1
        assert sl < NSLOT
        return sl

    # threshold pairs
    tps = []
    k = 0
    while k < u:
        tps.append((k, k + 1) if k + 1 < u else (k, None))
        k += 2
    assert len(tps) == n_tp

    # ---- hist work queue (ops on xf [128,2048]), balanced by cost ----
    hist_queue = []    # ("dve", ea, eb) or ("act", ea)
    hl = {"dve": 0.0, "act": 0.0}
    ei = 0
    while ei < E:
        if ei + 1 < E and hl["dve"] + dve_cost(2048) <= hl["act"] + 2 * act_cost(2048):
            hist_queue.append(("dve", ei, ei + 1))
            hl["dve"] += dve_cost(2048)
            ei += 2
        elif hl["act"] + act_cost(2048) <= hl["dve"] + dve_cost(2048):
            hist_queue.append(("act", ei, None))
            hl["act"] += act_cost(2048)
            ei += 1
        else:
            hist_queue.append(("dve", ei, None))
            hl["dve"] += dve_cost(2048)
            ei += 1

    with tile.TileContext(nc) as tc:
        import contextlib
        ctx = contextlib.ExitStack()
        with ctx:
            const_p = ctx.enter_context(tc.tile_pool(name="const", bufs=1))
            acc = const_p.tile([128, NSLOT], f32)
            nc.vector.memset(acc[:], 0.0)
            cik = const_p.tile([128, n_runs * 4 * u], f32)
            nc.sync.dma_start(cik[:], dCIK[:])
            xf = const_p.tile([128, 2048], f32)
            nc.sync.dma_start(xf[:], dXF[:])
            hed = const_p.tile([128, max(E, 1)], f32)
            nc.sync.dma_start(hed[:], dHED[:])
            bth = const_p.tile([128, max(NG, 1)], f32)
            nc.sync.dma_start(bth[:], dBTH[:])
            ones2 = const_p.tile([2, 128], bt)
            nc.vector.memset(ones2[:], 1.0)
            scr = const_p.tile([128, 2048], bt)     # dve scratch out
            scrf = const_p.tile([128, 2048], f32)   # act scratch out
            bx0 = const_p.tile([128, 1024], bt)
            nc.sync.dma_start(bx0[:], dBX[0])
            bx1 = const_p.tile([128, 1024], bt)
            nc.sync.dma_start(bx1[:], dBX[1])
            if MTOT > 0:
                pm0 = const_p.tile([128, MTOT], bt)
                nc.sync.dma_start(pm0[:], dPM[0, :, 0:MTOT])
                pm1 = const_p.tile([128, MTOT], bt)
                nc.sync.dma_start(pm1[:], dPM[1, :, 0:MTOT])

            psum_p = ctx.enter_context(
                tc.tile_pool(name="cps", bufs=2, space="PSUM"))
            lp = ctx.enter_context(tc.tile_pool(name="lp", bufs=3))
            rp = ctx.enter_context(tc.tile_pool(name="rp", bufs=3))
            np_ = ctx.enter_context(tc.tile_pool(name="nsqp", bufs=3))

            def emit_hist_one():
                if not hist_queue:
                    return
                kind, ea, eb = hist_queue.pop(0)
                slot = new_slot()
                if kind == "dve":
                    s1v = hed[:, eb:eb + 1] if eb is not None else BIG
                    nc.vector._custom_dve(
                        CNT2, out=scr[:, 0:2048], in0=xf[:],
                        s0=hed[:, ea:ea + 1], s1=s1v,
                        imm2=B_PACK, accum_out=acc[:, slot:slot + 1])
                else:
                    nc.scalar.activation(
                        scrf[:, 0:2048], xf[:], AT.Sign,
                        bias=hed[:, ea:ea + 1], scale=-1.0,
                        accum_out=acc[:, slot:slot + 1])
                meta["hist"].append((kind, slot, ea, eb, 2048))

            # ---- box groups through the same psum pipeline ----
            g0 = 0
            for g, mg in enumerate(box_groups):
                pg = psum_p.tile([128, 2048], f32, tag="pg")
                for nsl in range(2):
                    nc.tensor.matmul(
                        pg[0:mg, nsl * 512:(nsl + 1) * 512],
                        pm0[:, g0:g0 + mg],
                        bx0[:, nsl * 512:(nsl + 1) * 512],
                        start=True, stop=False)
                    nc.tensor.matmul(
                        pg[0:mg, nsl * 512:(nsl + 1) * 512],
                        pm1[:, g0:g0 + mg],
                        bx1[:, nsl * 512:(nsl + 1) * 512],
                        start=False, stop=True)
                slot = new_slot()
                # count(pooled > theta): sign(theta - pooled) -> (w - sum)/2
                nc.scalar.activation(
                    scrf[0:mg, 0:1024], pg[0:mg, 0:1024], AT.Sign,
                    bias=bth[0:mg, g:g + 1], scale=-1.0,
                    accum_out=acc[0:mg, slot:slot + 1])
                meta["box"].append((slot, g, mg, 1024))
                g0 += mg
                emit_hist_one()

            # ---- cdist runs ----
            pair_slot = 0
            for ri, rl in enumerate(run_struct):
                w = rl * BLK
                l0 = lp.tile([128, BLK], bt, tag="l0")
                nc.gpsimd.dma_start(l0[:], dL[0, ri])
                l1 = lp.tile([128, BLK], bt, tag="l1")
                nc.gpsimd.dma_start(l1[:], dL[1, ri])
                r0 = rp.tile([128, 2048], bt, tag="r0")
                nc.gpsimd.dma_start(r0[:, 0:w], dR[0, ri, :, 0:w])
                r1 = rp.tile([128, 2048], bt, tag="r1")
                nc.gpsimd.dma_start(r1[:, 0:w], dR[1, ri, :, 0:w])
                nst = np_.tile([2, 2048], bt, tag="nst")
                nc.sync.dma_start(nst[:, 0:w], dNSQ[ri, :, 0:w])
                for r in range(4):
                    pg = psum_p.tile([128, 2048], f32, tag="pg")
                    for kt in range(2):
                        lt = (l0, l1)[kt]
                        rt = (r0, r1)[kt]
                        for j in range(rl):
                            nc.tensor.matmul(
                                pg[:, j * BLK:(j + 1) * BLK],
                                lt[:, r * 128:(r + 1) * 128],
                                rt[:, j * BLK:(j + 1) * BLK],
                                start=(kt == 0), stop=False)
                    for j in range(rl):
                        nc.tensor.matmul(
                            pg[:, j * BLK:(j + 1) * BLK],
                            ones2[:], nst[:, j * BLK:(j + 1) * BLK],
                            start=False, stop=True)
                    ops, _cost = plan_group(w, tps)
                    base = (ri * 4 + r) * u
                    for (eng, ka, kb, c0, c1) in ops:
                        slot = new_slot()
                        if eng == "dve":
                            cb = cik[:, base + kb:base + kb + 1]                                 if kb is not None else BIG
                            nc.vector._custom_dve(
                                CNT2, out=scr[:, 0:c1 - c0],
                                in0=pg[:, c0:c1],
                                s0=cik[:, base + ka:base + ka + 1],
                                s1=cb, imm2=B_PACK,
                                accum_out=acc[:, slot:slot + 1])
                            meta["cdist"].append(
                                ("dve", slot, ri, r, ka, kb, c1 - c0))
                        else:
                            nc.scalar.activation(
                                scrf[:, 0:c1 - c0], pg[:, c0:c1], AT.Sign,
                                bias=cik[:, base + ka:base + ka + 1],
                                scale=-1.0,
                                accum_out=acc[:, slot:slot + 1])
                            meta["cdist"].append(
                                ("act", slot, ri, r, ka, None, c1 - c0))
                            if kb is not None:
                                slot = new_slot()
                                nc.scalar.activation(
                                    scrf[:, 0:c1 - c0], pg[:, c0:c1],
                                    AT.Sign,
                                    bias=cik[:, base + kb:base + kb + 1],
                                    scale=-1.0,
                                    accum_out=acc[:, slot:slot + 1])
                                meta["cdist"].append(
                                    ("act", slot, ri, r, kb, None, c1 - c0))
                    emit_hist_one()
                pair_slot += rl

            while hist_queue:
                emit_hist_one()

            nc.sync.dma_start(dOUT[:], acc[:])

    nc.compile()
    return nc, meta


# --------------------------------------------------------------------------
# host orchestration
# --------------------------------------------------------------------------
def kernel(x, scale_params, scale_importance):
    from concourse.bass_utils import run_bass_kernel_spmd

    x = np.asarray(x, dtype=np.float32)
    scale_params = np.asarray(scale_params, dtype=np.float32)
    scale_importance = np.asarray(scale_importance, dtype=np.float32)
    n, d = x.shape
    assert (n, d) == (N_ROWS, DIM)

    x64 = x.astype(np.float64)
    # ---- dynamic scales (mirror reference host-side computation) ----
    s = np.exp(scale_params.astype(np.float64))
    std_factor = float(x64.std(ddof=1) / x64.mean())
    std_factor = min(max(std_factor, 0.5), 2.0)
    adj = np.clip(s * std_factor, 2.0, 16.0)
    scales = [int(v) for v in adj]
    log_s = np.log(np.asarray(scales, np.float32)).astype(np.float64)

    # ---- derived constants ----
    uniq_scales = sorted(set(scales))
    uniq_t = sorted(set(float(ss) * float(ss) for ss in scales))
    u = len(uniq_t)
    n_tp = (u + 1) // 2

    # box: theta per unique scale; pooling matrix columns
    box_cols = []   # list of (scale, block_index)
    thetas = {}
    for ss in uniq_scales:
        m = d // ss
        nn = m * ss
        thetas[ss] = float(x64[:, :nn].sum() / (n * nn))
        for b in range(m):
            box_cols.append((ss, b))
    MTOT = len(box_cols)
    box_groups = []
    rem = MTOT
    while rem > 0:
        g = min(128, rem)
        box_groups.append(g)
        rem -= g
    NG = len(box_groups)

    # hist: deduped interior edges (f32 linspace like jnp.histogram)
    xmin = float(x.min())
    xmax = float(x.max())
    edge_list = []      # deduped values
    edge_map = {}       # (scale, k) -> index into edge_list
    for ss in uniq_scales:
        ed = np.linspace(np.float32(xmin), np.float32(xmax), ss + 1,
                         dtype=np.float32)
        for kk in range(1, ss):
            v = float(ed[kk])
            if v not in edge_map:
                edge_map[v] = len(edge_list)
                edge_list.append(v)
            edge_map[(ss, kk)] = edge_map[v]
    E = len(edge_list)

    run_struct_off, runs_per_core = _plan_runs()
    run_struct = tuple(list(run_struct_off) + [1, 1])

    cfg_key = (u, n_tp, E, tuple(box_groups), run_struct, MTOT)
    if cfg_key not in _BUILD_CACHE:
        _BUILD_CACHE[cfg_key] = _build(
            cfg_key, u, n_tp, edge_list, box_groups, run_struct)
    nc, meta = _BUILD_CACHE[cfg_key]

    # ---- per-core inputs ----
    xb = x.astype(bf16)                       # [8192, 256]
    xTb = np.ascontiguousarray(xb.T)          # [256, 8192]
    sq = (x.astype(np.float32) ** 2).sum(axis=1, dtype=np.float32)  # [8192]
    nsq_half = -0.5 * sq
    nsq_hi = nsq_half.astype(bf16)
    nsq_lo = (nsq_half - nsq_hi.astype(np.float32)).astype(bf16)

    n_runs = len(run_struct)
    n_pairs = 17

    # pooling matrix [256, MTOT] bf16 (same all cores)
    PM = np.zeros((256, max(MTOT, 1)), np.float32)
    for col, (ss, b) in enumerate(box_cols):
        PM[b * ss:(b + 1) * ss, col] = 1.0 / ss
    PM_b = PM.astype(bf16)
    dPM = np.stack([PM_b[0:128], PM_b[128:256]])          # [2,128,MTOT]
    dBTH = np.zeros((128, max(NG, 1)), np.float32)
    g0 = 0
    for g, mg in enumerate(box_groups):
        for p in range(mg):
            ss, b = box_cols[g0 + p]
            dBTH[p, g] = thetas[ss]
        g0 += mg

    t_arr = np.asarray(uniq_t, np.float64)
    dHED_np = np.zeros((128, max(E, 1)), np.float32)
    for ei2, ev in enumerate(edge_list):
        dHED_np[:, ei2] = ev

    in_maps = []
    core_meta = []
    for c in range(NCORES):
        runs = runs_per_core[c]
        L = np.zeros((2, n_runs, 128, BLK), bf16)
        R = np.zeros((2, n_runs, 128, 2048), bf16)
        NSQ = np.zeros((n_runs, 2, 2048), bf16)
        CIK = np.zeros((128, n_runs * 4 * u), np.float32)
        ps = 0
        pair_list = []
        for ri, (a, bs, is_diag) in enumerate(runs):
            for kt in range(2):
                L[kt, ri] = xTb[kt * 128:(kt + 1) * 128,
                                a * BLK:(a + 1) * BLK]
            for j, b in enumerate(bs):
                for kt in range(2):
                    R[kt, ri, :, j * BLK:(j + 1) * BLK] = xTb[
                        kt * 128:(kt + 1) * 128, b * BLK:(b + 1) * BLK]
                NSQ[ri, 0, j * BLK:(j + 1) * BLK] = nsq_hi[b * BLK:(b + 1) * BLK]
                NSQ[ri, 1, j * BLK:(j + 1) * BLK] = nsq_lo[b * BLK:(b + 1) * BLK]
            for r in range(4):
                i0 = a * BLK + r * 128
                sqi = sq[i0:i0 + 128].astype(np.float64)
                for kk in range(u):
                    CIK[:, (ri * 4 + r) * u + kk] = (
                        (sqi - t_arr[kk]) * 0.5).astype(np.float32)
            pair_list.append((a, list(bs), is_diag))
            ps += len(bs)
        rows = x[c * 1024:(c + 1) * 1024]                  # own rows
        XF = np.ascontiguousarray(
            rows.reshape(8, 128, 256).transpose(1, 0, 2).reshape(128, 2048))
        rowsT_b = xTb[:, c * 1024:(c + 1) * 1024]
        BX = np.stack([rowsT_b[0:128], rowsT_b[128:256]])  # [2,128,1024]
        in_maps.append({
            "L": L, "R": R, "NSQ": NSQ, "CIK": CIK,
            "XF": np.ascontiguousarray(XF),
            "BX": np.ascontiguousarray(BX),
            "PM": dPM, "BTH": dBTH, "HED": dHED_np,
        })
        core_meta.append(pair_list)

    res = run_bass_kernel_spmd(nc, in_maps, core_ids=list(range(NCORES)))

    # ---- decode ----
    corr_counts = np.zeros(u, np.float64)
    box_counts = {ss: 0.0 for ss in uniq_scales}
    hist_cum = np.zeros(E, np.float64)

    for c in range(NCORES):
        out = res.results[c]["OUT"].astype(np.float64)   # [128, NSLOT]
        pair_list = core_meta[c]
        for ent in meta["cdist"]:
            kind, slot, ri, r, ka, kb, w = ent
            a, bs, is_diag = pair_list[ri]
            wt = 1.0 if is_diag else 2.0
            vals = out[:, slot]
            if kind == "dve":
                c1 = np.mod(vals, B_PACK)
                c2 = np.floor(vals / B_PACK)
                corr_counts[ka] += wt * c1.sum()
                if kb is not None:
                    corr_counts[kb] += wt * c2.sum()
            else:
                # count(v > c) = (w - sum_sign)/2 per partition
                corr_counts[ka] += wt * ((w - vals) / 2.0).sum()
        for (slot, g, mg, wbox) in meta["box"]:
            vals = out[0:mg, slot]
            cnt = (wbox - vals) / 2.0     # count(pooled > theta)
            gg0 = sum(box_groups[:g])
            for p in range(mg):
                ss, b = box_cols[gg0 + p]
                box_counts[ss] += cnt[p]
        for ent in meta["hist"]:
            kind, slot, ea, eb, wh = ent
            vals = out[:, slot]
            if kind == "dve":
                cgt1 = np.mod(vals, B_PACK).sum()
                cgt2 = np.floor(vals / B_PACK).sum()
                hist_cum[ea] += 2048 * 128 - cgt1   # count(x < e) = w - count(x > e)  (ties ~0)
                hist_cum[eb] += 2048 * 128 - cgt2
            else:
                hist_cum[ea] += ((wh + vals) / 2.0).sum()

    # ---- slope fits (host) ----
    def slope(xv, yv):
        xv = np.asarray(xv, np.float64)
        yv = np.asarray(yv, np.float64)
        xm = xv.mean()
        ym = yv.mean()
        dx = xv - xm
        with np.errstate(divide="ignore", invalid="ignore"):
            return float((dx * (yv - ym)).sum() / (dx * dx).sum())

    t_index = {t: i for i, t in enumerate(uniq_t)}
    corr_per_scale = np.array(
        [corr_counts[t_index[float(ss) * float(ss)]] for ss in scales])
    box_per_scale = np.array([box_counts[ss] for ss in scales])

    ents = []
    total = float(n * d)
    for ss in scales:
        cum = np.zeros(ss + 1, np.float64)
        cum[0] = 0.0
        cum[ss] = total
        for kk in range(1, ss):
            cum[kk] = hist_cum[edge_map[(ss, kk)]]
        hist = np.diff(cum)
        p = hist / total
        with np.errstate(divide="ignore", invalid="ignore"):
            ents.append(float(-(np.where(p > 0, p * np.log(
                np.where(p > 0, p, 1.0)), 0.0)).sum()))

    with np.errstate(divide="ignore", invalid="ignore"):
        box_dim = -slope(log_s, np.log(box_per_scale))
        corr_dim = slope(log_s, np.log(corr_per_scale))
    info_dim = slope(log_s, np.asarray(ents))

    # softmax in f32 like the reference
    si = scale_importance.astype(np.float64)
    w_ = np.exp(si - si.max())
    w_ = w_ / w_.sum()
    out_val = w_[0] * box_dim + w_[1] * corr_dim + w_[2] * info_dim
    return np.float32(out_val)
